# revision 17
# baseline (speedup 1.0000x reference)
"""KAN layer (piecewise-linear spline edges) as a Trainium2 Bass kernel.

Math: y[b,o] = sum_i lerp(S[o,i,:], u) + bias[o],  u = (clip(x[b,i]*W[o,i],-1,1)+1)*7.5

Transformation: each edge function f_{o,i}(x) is piecewise-linear in x; it is
resampled onto a shared uniform x-grid of GX points and decomposed into
relu ramps anchored at the grid knots plus an exact affine part:

    f(x) = alpha + beta*x + sum_h C[h] * ramp_h(x)
    ramp_h(x) = relu(x_h - x) for x_h < 0 (falling), relu(x - x_h) else

C = second differences of the resampled values (kink strengths).  Centered
(two-sided) ramps halve bf16 ramp magnitudes; the affine part runs as a
bf16 hi/lo matmul (exact to ~1e-4) and alpha sums are added host-side.
Then  y[b,o] = sum_{i,h} C[o,i,h]*ramp_h(x[b,i]) + affine  — a dense matmul
over K=(i,h) with ramps built on-chip in ONE elementwise op per chunk:
ACT:  relu(s_p*px + bias_p)   (per-partition scale/bias APs)
DVE:  max(px - x_h, 0)  or  min(px - x_h, 0) = -ramp (sign folded into table)

Sharding: 8 cores = 2 batch-groups x 4 in-feature-groups.  Each core: 512
batch rows x 64 in-features, all 256 outputs; host sums 4 partials per batch
group.  K layout per core: partition p = i_lo*8 + h_lo (i_lo 16, h_lo 8),
chunk j = h_hi, i_hi 0..3.  x is replicated across partitions by one K=128
matmul per i_hi whose 0/1 pattern also folds the bf16 hi+lo split of x
(hi rows 0-63, lo rows 64-127 of the stationary source).
Big matmul: stationary = table slice [K=128, o-half 128], moving = ramp tile
[K=128, b 512], PSUM output transposed y^T [o, b].

A burst of dummy matmuls at t0 (during input DMA) warms the PE HAM clock
gate (1.2 -> 2.4 GHz) before the real matmuls issue.
"""

import numpy as np
import ml_dtypes

import concourse.bacc as bacc
import concourse.bass as bass
import concourse.mybir as mybir
import concourse.tile as tile
from concourse.bass_utils import run_bass_kernel_spmd

B, IN, OUT, G = 1024, 256, 256, 16
GX = 32                 # shared x-grid size (emulated rel err ~9.6e-3, gate 2e-2)
NH = GX // 8            # h_hi chunk count
HALF = 16               # knots below HALF use falling ramps (chunk-aligned)
NFALL = HALF // 8       # number of falling chunks
NC_N = 8
NBG = 2                 # batch groups
NIG = 4                 # in-feature groups
BS = B // NBG           # 512 batch rows per core
ISH = IN // NIG         # 64 in-features per core
WARMUP_MM = 30          # dummy matmuls to flip the PE HAM clock gate
AF = np.dtype(ml_dtypes.bfloat16)

_PROG_CACHE = {}


def _build_program():
    nc = bacc.Bacc(
        "TRN2",
        target_bir_lowering=False,
        debug=False,
        enable_asserts=False,
        num_devices=NC_N,
    )
    f32 = mybir.dt.float32
    bf16 = mybir.dt.bfloat16
    Act = mybir.ActivationFunctionType
    Alu = mybir.AluOpType

    cst_d = nc.dram_tensor("cst", [128, BS + NIG * 128], bf16, kind="ExternalInput")
    abt_d = nc.dram_tensor("abt", [128, OUT], bf16, kind="ExternalInput")
    hba_d = nc.dram_tensor("hba", [128, NH], f32, kind="ExternalInput")
    hbb_d = nc.dram_tensor("hbb", [128, NH], f32, kind="ExternalInput")
    atab0_d = nc.dram_tensor("atab0", [128, 1024], bf16, kind="ExternalInput")
    atab1_d = nc.dram_tensor("atab1", [128, 1024], bf16, kind="ExternalInput")
    atabR_d = nc.dram_tensor("atabR", [128, (NH - 2) * 1024], bf16, kind="ExternalInput")
    y_d = nc.dram_tensor("y", [2, 128, 512], f32, kind="ExternalOutput")

    with tile.TileContext(nc) as tc:
        with (
            tc.tile_pool(name="const", bufs=1) as cp,
            tc.tile_pool(name="atp", bufs=NH) as atp,
            tc.tile_pool(name="pxpa", bufs=1, space="PSUM") as pxpa,
            tc.tile_pool(name="pxpb", bufs=1, space="PSUM") as pxpb,
            tc.tile_pool(name="pypa", bufs=1, space="PSUM") as pypa,
            tc.tile_pool(name="pypb", bufs=1, space="PSUM") as pypb,
            tc.tile_pool(name="pwp", bufs=1, space="PSUM") as pwp,
            tc.tile_pool(name="htpa", bufs=NH) as hpa,
            tc.tile_pool(name="htpb", bufs=NH) as hpb,
        ):
            # ---- input DMAs first (latency ~2.5us); combined const tensor
            cst = cp.tile([128, BS + NIG * 128], bf16)
            nc.scalar.dma_start(cst, cst_d.ap())
            abt = cp.tile([128, OUT], bf16)
            nc.gpsimd.dma_start(abt, abt_d.ap())
            hba = cp.tile([128, NH], f32)
            nc.sync.dma_start(hba, hba_d.ap())
            hbb = cp.tile([128, NH], f32)
            nc.sync.dma_start(hbb, hbb_d.ap())
            at0 = atp.tile([128, 1024], bf16, tag="at0")
            nc.gpsimd.dma_start(at0, atab0_d.ap())
            at1 = atp.tile([128, 1024], bf16, tag="at1")
            nc.gpsimd.dma_start(at1, atab1_d.ap())
            atR = atp.tile([128, (NH - 2) * 1024], bf16, tag="atR")
            nc.sync.dma_start(atR, atabR_d.ap())
            xt2 = cst[:, 0:BS]
            pats = cst[:, BS:BS + NIG * 128]

            # ---- PE warmup burst: garbage matmuls into a scratch PSUM bank
            wsrc = cp.tile([128, 128], bf16)
            nc.vector.memset(wsrc, 0)
            wps = pwp.tile([128, 128], f32)
            for _ in range(WARMUP_MM):
                nc.tensor.matmul(wps, lhsT=wsrc, rhs=wsrc,
                                 start=True, stop=True, skip_group_check=True)
            # preload the Relu ACT table during warmup (off critical path)
            wact = cp.tile([128, 8], bf16)
            nc.scalar.activation(wact, wsrc[:, 0:8], Act.Relu, bias=0.0, scale=1.0)

            # ---- replicate x across partitions: px[p, q*BS+b] = x[i(q,p), b]
            # two separate PSUM tiles so the ACT and DVE ramp readers are
            # fully independent (shared tiles serialize cross-engine)
            pxa = pxpa.tile([128, 2 * BS], f32)
            pxb = pxpb.tile([128, 2 * BS], f32)
            for q in range(NIG):
                dst = pxa if q < 2 else pxb
                nc.tensor.matmul(
                    dst[:, (q % 2) * BS:(q % 2 + 1) * BS],
                    lhsT=pats[:, q * 128:(q + 1) * 128],
                    rhs=xt2,
                    start=True, stop=True, skip_group_check=True,
                )

            # ---- affine part: pyT[o,b] += beta-table^T @ [xhi; xlo]
            pyTa = pypa.tile([128, 512], f32)
            pyTb = pypb.tile([128, 512], f32)
            for oh in range(2):
                nc.tensor.matmul(
                    pyTa if oh == 0 else pyTb,
                    lhsT=abt[:, oh * 128:(oh + 1) * 128],
                    rhs=xt2,
                    start=True, stop=False, skip_group_check=True,
                )

            # ---- ramp chunks + accumulating matmuls
            # each chunk is computed half by ACT (cols 0:1024, true ramp) and
            # half by DVE (cols 1024:2048; min() for falling chunks gives the
            # NEGATED ramp -- sign folded into that half's table columns)
            for j in range(NH):
                falling = j < NFALL
                hta = hpa.tile([128, 2 * BS], bf16, tag="hta")
                htb = hpb.tile([128, 2 * BS], bf16, tag="htb")
                nc.scalar.activation(
                    hta, pxa, Act.Relu,
                    bias=hba[:, j:j + 1], scale=(-1.0 if falling else 1.0))
                nc.vector.tensor_scalar(
                    htb, pxb,
                    hbb[:, j:j + 1], 0.0,
                    Alu.subtract, Alu.min if falling else Alu.max)
                # last chunk: all oh=0 matmuls first so the first y-half
                # can drain while the oh=1 matmuls still run
                order = ([(ih, oh) for ih in range(NIG) for oh in range(2)]
                         if j < NH - 1 else
                         [(ih, oh) for oh in range(2) for ih in range(NIG)])
                for ih, oh in order:
                    src_t = hta if ih < 2 else htb
                    off = (ih % 2) * BS
                    if j == 0:
                        lhsT = at0[:, (ih * 2 + oh) * 128:(ih * 2 + oh + 1) * 128]
                    elif j == 1:
                        lhsT = at1[:, (ih * 2 + oh) * 128:(ih * 2 + oh + 1) * 128]
                    else:
                        lhsT = atR[:, (j - 2) * 1024 + (ih * 2 + oh) * 128:
                                   (j - 2) * 1024 + (ih * 2 + oh + 1) * 128]
                    nc.tensor.matmul(
                        pyTa if oh == 0 else pyTb,
                        lhsT=lhsT,
                        rhs=src_t[:, off:off + BS],
                        start=False,
                        stop=(j == NH - 1 and ih == NIG - 1),
                        skip_group_check=True,
                    )

            # ---- drain y^T and store (parallel halves, parallel DMA queues)
            ysba = cp.tile([128, 512], f32)
            ysbb = cp.tile([128, 512], f32)
            nc.vector.tensor_copy(ysba, pyTa)
            nc.scalar.copy(ysbb, pyTb)
            nc.sync.dma_start(y_d.ap()[0], ysba)
            nc.scalar.dma_start(y_d.ap()[1], ysbb)

    nc.compile()
    return nc


def _edge_table(W, S, bias, xs):
    """PHI[o,i,h] = edge function at grid xs (float64), bias folded in."""
    Wf = W.reshape(-1, 1).astype(np.float64)
    Sf = S.reshape(-1, G).astype(np.float64)
    tt = np.clip(Wf * xs[None, :], -1.0, 1.0)
    uu = (tt + 1.0) * (0.5 * (G - 1))
    idx = np.clip(np.floor(uu).astype(np.int64), 0, G - 2)
    frac = uu - idx
    ar = np.arange(Sf.shape[0])[:, None]
    phi = Sf[ar, idx] + frac * (Sf[ar, idx + 1] - Sf[ar, idx])
    phi = phi.reshape(OUT, IN, len(xs))
    phi += bias.astype(np.float64)[:, None, None] / IN
    return phi


def kernel(x, W, spline_values, bias, _trace=False):
    x = np.asarray(x, dtype=np.float32)
    W = np.asarray(W, dtype=np.float32)
    S = np.asarray(spline_values, dtype=np.float32)
    bias = np.asarray(bias, dtype=np.float32)

    xmax = np.float32(float(np.abs(x).max()) * (1.0 + 1e-6) + 1e-30)
    dx = np.float32(2.0 * float(xmax) / (GX - 1))
    xh = (np.arange(GX, dtype=np.float32) * dx - xmax).astype(np.float64)
    phi = _edge_table(W, S, bias, xh)

    # kink strengths; edge knots carry none
    C = np.zeros((OUT, IN, GX))
    C[:, :, 1:GX - 1] = (phi[:, :, 2:] - 2 * phi[:, :, 1:GX - 1] + phi[:, :, :GX - 2]) / np.float64(dx)
    # affine part: residual at the two grid ends
    r0 = phi[:, :, 0] - np.einsum('oih,h->oi', C[:, :, 1:HALF], xh[1:HALF] - xh[0])
    r1 = phi[:, :, -1] - np.einsum('oih,h->oi', C[:, :, HALF:GX - 1], xh[-1] - xh[HALF:GX - 1])
    beta = (r1 - r0) / (xh[-1] - xh[0])
    alpha = r0 - beta * xh[0]
    A2 = alpha.sum(axis=1).astype(np.float64)          # [OUT], added host-side
    bhi = beta.astype(AF).astype(np.float64)
    blo = (beta - bhi).astype(AF)
    co1 = (bhi + blo.astype(np.float64)).astype(AF)    # vs xhi rows
    co2 = beta.astype(AF)                              # vs xlo rows

    p_idx = np.arange(128)
    i_lo = p_idx // 8
    h_lo = p_idx % 8

    # table sign: DVE half (i_hi 2,3) of falling chunks uses min() = -ramp
    atabs_by_ig = []
    for ig in range(NIG):
        a = np.empty((NH, 128, NIG, OUT), np.float64)
        for j in range(NH):
            h = j * 8 + h_lo
            for ih in range(NIG):
                sgn = -1.0 if (ih >= 2 and j < NFALL) else 1.0
                i_g = ig * ISH + ih * 16 + i_lo
                a[j, :, ih, :] = sgn * C[:, i_g, h].T
        atabs_by_ig.append(np.ascontiguousarray(a.reshape(NH, 128, NIG * OUT)).astype(AF))

    # affine stationary: rows 0-63 (xhi) -> co1, rows 64-127 (xlo) -> co2
    abts = []
    for ig in range(NIG):
        ab = np.zeros((128, OUT), np.float32)
        ab[:64] = co1[:, ig * ISH:(ig + 1) * ISH].T.astype(np.float32)
        ab[64:] = co2[:, ig * ISH:(ig + 1) * ISH].T.astype(np.float32)
        abts.append(ab.astype(AF))

    # replication pattern: pats[k, q*128+m] = 1 at k=q*16+m//8 and 64+q*16+m//8
    pats = np.zeros((128, NIG * 128), np.float32)
    m = np.arange(128)
    for q in range(NIG):
        pats[q * 16 + m // 8, q * 128 + m] = 1.0
        pats[64 + q * 16 + m // 8, q * 128 + m] = 1.0
    pats = pats.astype(AF)

    # per-partition ramp params
    hba = np.zeros((128, NH), np.float32)
    hbb = np.zeros((128, NH), np.float32)
    for j in range(NH):
        h = j * 8 + h_lo
        xhj = (h.astype(np.float32) * dx - xmax)
        s = np.where(h < HALF, np.float32(-1.0), np.float32(1.0))
        hba[:, j] = -s * xhj         # ACT bias
        hbb[:, j] = xhj              # DVE subtract operand
    in_maps = []
    for c in range(NC_N):
        bg, ig = c // NIG, c % NIG
        xs = x[bg * BS:(bg + 1) * BS, ig * ISH:(ig + 1) * ISH].T  # [64, BS] f32
        xhi = xs.astype(AF)
        xlo = (xs - xhi.astype(np.float32)).astype(AF)
        cst = np.zeros((128, BS + NIG * 128), AF)
        cst[:64, 0:BS] = xhi
        cst[64:128, 0:BS] = xlo
        cst[:, BS:BS + NIG * 128] = pats
        at = atabs_by_ig[ig]
        in_maps.append({
            "cst": cst,
            "abt": abts[ig],
            "hba": hba,
            "hbb": hbb,
            "atab0": at[0],
            "atab1": at[1],
            "atabR": np.ascontiguousarray(
                at[2:].transpose(1, 0, 2).reshape(128, (NH - 2) * 1024)),
        })

    key = "prog"
    if key not in _PROG_CACHE:
        _PROG_CACHE[key] = _build_program()
    nc = _PROG_CACHE[key]

    res = run_bass_kernel_spmd(
        nc, in_maps, core_ids=list(range(NC_N)), trace=bool(_trace)
    )
    # y_core [128, 1024]: [p, oh*512 + b] = y^T[oh*128+p, b]; sum over ig
    y = np.empty((B, OUT), np.float32)
    for bg in range(NBG):
        acc = np.zeros((OUT, BS), np.float64)
        for ig in range(NIG):
            a = res.results[bg * NIG + ig]["y"]
            acc += a.reshape(256, 512)
        acc += A2[:, None]
        y[bg * BS:(bg + 1) * BS] = acc.T.astype(np.float32)
    if _trace:
        kernel._last_result = res
    return y


if __name__ == "__main__":
    rng = np.random.default_rng(0)
    x = rng.standard_normal((B, IN)).astype(np.float32)
    W = (rng.uniform(-1, 1, (OUT, IN)) / np.sqrt(IN)).astype(np.float32)
    S = rng.standard_normal((OUT, IN, G)).astype(np.float32)
    b = np.zeros(OUT, np.float32)
    y = kernel(x, W, S, b)
    print("y", y.shape, y.dtype)
